# revision 1
# baseline (speedup 1.0000x reference)
"""GATv2 graph layer Bass kernel for TRN2 (SPMD across NeuronCores, no
collectives).

Design: edges sorted by destination node and sharded across cores by dst
range. Each core builds two fp16 gather tables in its DRAM:
  xsrc[n]        = node_emb @ W_src                      [N, 128]
  xdext[n*8+t]   = node_emb @ W_dst + edge_emb_eff[t]    [NPC*8, 128]
Per edge chunk (128 edges, single 128-node dst window, single src-table
half): dma_gather xs and xd rows, comb = lrelu(xs + xd), transpose via
TensorE, logits = comb_T^T @ att_blk, ex = exp(logits) (no max-subtraction:
logits are bounded, exp stays finite in f32), weighted = xs * bcast32(ex),
then a one-hot scatter matmul accumulates [sum(ex) | sum(ex*xs)] per dst
node in PSUM. Window flush: agg = sum(ex*xs)/sum(ex), @W_out (gamma-scaled)
+ beta_eff, residual, LayerNorm, DMA out.
"""
import numpy as np
from contextlib import ExitStack
from dataclasses import dataclass

import concourse.bass as bass
import concourse.tile as tile
from concourse import bacc, mybir
from concourse.masks import make_identity

P = 128
HID = 128
H = 4
HD = 32
NET = 8
EPS_LN = 1e-5
MAXCALL = 4096  # max idxs per dma_gather call
DEAD = -5.0     # dst_rel for padding slots (matches no window column)


@dataclass
class Geo:
    N: int
    n_cores: int
    split: int          # src-table row split (must be <= 32767)
    slab_w: int = 5     # windows per slab

    @property
    def npc(self):
        return self.N // self.n_cores

    @property
    def nw(self):
        return (self.npc + P - 1) // P

    @property
    def nslab(self):
        return (self.nw + self.slab_w - 1) // self.slab_w

    @property
    def n_pad(self):   # nodeT padded cols
        return ((self.N + 511) // 512) * 512


def wrap_idx(idx, cols):
    n = idx.shape[0]
    assert n % 16 == 0
    w = np.zeros((P, cols), dtype=np.int16)
    if n:
        t16 = idx.reshape(n // 16, 16).T
        for g in range(8):
            w[g * 16:(g + 1) * 16, :n // 16] = t16
    return w


def host_prep(g: Geo, node_embeddings, edge_index, edge_type, task_embedding,
              W_src, b_src, W_dst, b_dst, edge_emb, att,
              W_out, b_out, norm_w, norm_b, W_film, b_film):
    """Returns (sched, in_maps). Pure index work + tiny constant folding;
    all O(N*HID) / O(E*HID) float math runs on device."""
    src = np.asarray(edge_index[0], dtype=np.int64)
    dst = np.asarray(edge_index[1], dtype=np.int64)
    et = np.asarray(edge_type, dtype=np.int64)
    npc = g.npc

    order = np.argsort(dst, kind="stable")
    src, dst, et = src[order], dst[order], et[order]
    core_of = dst // npc

    buckets = {}
    for c in range(g.n_cores):
        m = core_of == c
        cs, cd, ce = src[m], dst[m] - c * npc, et[m]
        for w in range(g.nw):
            wm = (cd // P) == w
            ws_, wd, we = cs[wm], cd[wm] - w * P, ce[wm]
            lo = ws_ < g.split
            buckets[(c, w, 0)] = (ws_[lo], wd[lo], we[lo])
            buckets[(c, w, 1)] = (ws_[~lo] - g.split, wd[~lo], we[~lo])

    caps = np.zeros((g.nw, 2), dtype=np.int64)
    for w in range(g.nw):
        for h in range(2):
            mx = max(len(buckets[(c, w, h)][0]) for c in range(g.n_cores))
            caps[w, h] = (mx + P - 1) // P

    # ---- schedule ---------------------------------------------------------
    sched_slabs = []
    total_chunks = 0
    for s in range(g.nslab):
        ws = list(range(s * g.slab_w, min((s + 1) * g.slab_w, g.nw)))
        chunks = []            # (win_local, half, slot)
        calls = {0: [], 1: [], 2: []}
        slot = 0
        for h in (0, 1):
            run = 0
            run_start = slot
            for w in ws:
                for _ in range(caps[w, h]):
                    chunks.append((w - ws[0], h, slot))
                    slot += 1
                    run += P
                    if run == MAXCALL:
                        calls[h].append((run_start, run))
                        run, run_start = 0, slot
            if run:
                calls[h].append((run_start, run))
        t = 0
        while t < slot * P:
            n = min(MAXCALL, slot * P - t)
            calls[2].append((t // P, n))
            t += n
        sched_slabs.append(dict(windows=ws, chunks=chunks, calls=calls,
                                chunk0=total_chunks))
        total_chunks += len(chunks)

    lo_cols = max(16, sum(n for sl in sched_slabs
                          for (_, n) in sl["calls"][0]) // 16)
    hi_cols = max(16, sum(n for sl in sched_slabs
                          for (_, n) in sl["calls"][1]) // 16)

    # ---- shared constants -------------------------------------------------
    nodeT = np.zeros((HID, g.n_pad), dtype=np.float16)
    nodeT[:, :g.N] = np.asarray(node_embeddings, np.float32).T.astype(np.float16)
    emb_eff = (np.asarray(edge_emb, np.float64)
               + np.asarray(b_src, np.float64)[None, :]
               + np.asarray(b_dst, np.float64)[None, :]).astype(np.float16)
    att_blk = np.zeros((HID, H), dtype=np.float16)
    for h in range(H):
        att_blk[h * HD:(h + 1) * HD, h] = np.asarray(att, np.float32)[h]

    consts = dict(
        nodeT=nodeT,
        W_src=np.asarray(W_src, np.float32).astype(np.float16),
        W_dst=np.asarray(W_dst, np.float32).astype(np.float16),
        W_out=np.asarray(W_out, np.float32).astype(np.float16),
        W_film=np.asarray(W_film, np.float32).astype(np.float16),
        b_film=np.asarray(b_film, np.float32).reshape(1, 2 * HID),
        b_out=np.asarray(b_out, np.float32).reshape(1, HID),
        task=np.asarray(task_embedding, np.float32).reshape(HID, 1)
            .astype(np.float16),
        emb_eff=emb_eff.reshape(1, NET * HID),
        att_blk=att_blk,
    )
    skip_norm = bool(np.all(np.asarray(norm_w) == 1.0)
                     and np.all(np.asarray(norm_b) == 0.0))
    if not skip_norm:
        consts["normw"] = np.asarray(norm_w, np.float32).reshape(1, HID)
        consts["normb"] = np.asarray(norm_b, np.float32).reshape(1, HID)

    # ---- per-core arrays --------------------------------------------------
    in_maps = []
    for c in range(g.n_cores):
        lo_l, hi_l, xd_l = [], [], []
        dstr = np.full((P, total_chunks), DEAD, dtype=np.float32)
        ci = 0
        for sl in sched_slabs:
            ws0 = sl["windows"][0]
            per_half = {0: [], 1: []}
            nth = {}
            for (wl, h, slot) in sl["chunks"]:
                w = ws0 + wl
                es, ed, ee = buckets[(c, w, h)]
                k = nth.get((wl, h), 0)
                nth[(wl, h)] = k + 1
                sl_src = np.zeros(P, dtype=np.int64)
                sl_dst = np.full(P, DEAD, dtype=np.float64)
                sl_et = np.zeros(P, dtype=np.int64)
                n = min(P, max(0, len(es) - k * P))
                if n > 0:
                    sl_src[:n] = es[k * P:k * P + n]
                    sl_dst[:n] = ed[k * P:k * P + n]
                    sl_et[:n] = ee[k * P:k * P + n]
                per_half[h].append(sl_src)
                xd_l.append(np.where(sl_dst == DEAD, 0,
                                     (sl_dst + wl * P) * NET + sl_et)
                            .astype(np.int64))
                dstr[:, ci] = sl_dst
                ci += 1
            lo_l.extend(per_half[0])
            hi_l.extend(per_half[1])
        lo_i = (np.concatenate(lo_l) if lo_l else np.zeros(0, np.int64))
        hi_i = (np.concatenate(hi_l) if hi_l else np.zeros(0, np.int64))
        xd_i = np.concatenate(xd_l) if xd_l else np.zeros(0, np.int64)
        assert lo_i.max(initial=0) < g.split <= 32767
        assert hi_i.max(initial=0) < 32768
        assert xd_i.max(initial=0) < g.slab_w * P * NET

        m = dict(consts)
        m["node_own"] = np.ascontiguousarray(
            np.asarray(node_embeddings, np.float32)[c * npc:(c + 1) * npc])
        m["lo_idx"] = wrap_idx(lo_i.astype(np.int16), lo_cols)
        m["hi_idx"] = wrap_idx(hi_i.astype(np.int16), hi_cols)
        m["xd_idx"] = wrap_idx(xd_i.astype(np.int16),
                               max(16, total_chunks * P // 16))
        m["dstr"] = dstr
        in_maps.append(m)

    sched = dict(slabs=sched_slabs, caps=caps, total_chunks=total_chunks,
                 lo_cols=lo_cols, hi_cols=hi_cols, skip_norm=skip_norm)
    return sched, in_maps


def build_program(g: Geo, sched, debug=False, ablate=()):
    nc = bacc.Bacc("TRN2", target_bir_lowering=False, debug=False,
                   num_devices=g.n_cores, num_swdge_queues=4)
    f16, f32 = mybir.dt.float16, mybir.dt.float32
    AF = mybir.ActivationFunctionType
    OP = mybir.AluOpType
    npc, nw = g.npc, g.nw
    total_chunks = sched["total_chunks"]
    lo_cols, hi_cols = sched["lo_cols"], sched["hi_cols"]
    xd_cols = max(16, total_chunks * P // 16)

    def din(name, shape, dt):
        return nc.dram_tensor(name, shape, dt, kind="ExternalInput").ap()

    nodeT = din("nodeT", [HID, g.n_pad], f16)
    node_own = din("node_own", [npc, HID], f32)
    W_src = din("W_src", [HID, HID], f16)
    W_dst = din("W_dst", [HID, HID], f16)
    W_out = din("W_out", [HID, HID], f16)
    W_film = din("W_film", [HID, 2 * HID], f16)
    b_film = din("b_film", [1, 2 * HID], f32)
    b_out = din("b_out", [1, HID], f32)
    task = din("task", [HID, 1], f16)
    emb_eff = din("emb_eff", [1, NET * HID], f16)
    att_blk = din("att_blk", [HID, H], f16)
    lo_idx = din("lo_idx", [P, lo_cols], mybir.dt.int16)
    hi_idx = din("hi_idx", [P, hi_cols], mybir.dt.int16)
    xd_idx = din("xd_idx", [P, xd_cols], mybir.dt.int16)
    dstr = din("dstr", [P, total_chunks], f32)
    out = nc.dram_tensor("out", [npc, HID], f32, kind="ExternalOutput").ap()

    xsrc_tab = nc.dram_tensor("xsrc_tab", [g.n_pad, HID], f16).ap()
    xd_tabs = [nc.dram_tensor(f"xd_tab{i}",
                              [min(g.slab_w, nw - i * g.slab_w) * P * NET, HID],
                              f16).ap()
               for i in range(g.nslab)]
    if debug:
        cmax = max(len(sl["chunks"]) for sl in sched["slabs"])
        nsl = len(sched["slabs"])
        dbg_xs = nc.dram_tensor("dbg_xs", [nsl, P, cmax, HID], f16,
                                kind="ExternalOutput").ap()
        dbg_xd = nc.dram_tensor("dbg_xd", [nsl, P, cmax, HID], f16,
                                kind="ExternalOutput").ap()
        dbg_comb = nc.dram_tensor("dbg_comb", [nsl, P, cmax, HID], f16,
                                  kind="ExternalOutput").ap()
        dbg_ex = nc.dram_tensor("dbg_ex", [nsl, P, cmax, 4], f16,
                                kind="ExternalOutput").ap()
        dbg_wgt = nc.dram_tensor("dbg_wgt", [nsl, P, cmax, HID], f16,
                                 kind="ExternalOutput").ap()
        dbg_win = nc.dram_tensor("dbg_win", [nw, P, 4 + HID], f32,
                                 kind="ExternalOutput").ap()
        dbg_noT = nc.dram_tensor("dbg_noT", [nw, HID, P], f16,
                                 kind="ExternalOutput").ap()
        dbg_no16 = nc.dram_tensor("dbg_no16", [nw, P, HID], f16,
                                  kind="ExternalOutput").ap()
        dbg_y = nc.dram_tensor("dbg_y", [nw, P, HID], f32,
                               kind="ExternalOutput").ap()
        nsl2 = len(sched["slabs"])
        dbg_combT = nc.dram_tensor("dbg_combT", [nsl2, HID, cmax, P], f16,
                                   kind="ExternalOutput").ap()
        dbg_aggn = nc.dram_tensor("dbg_aggn", [nw, P, HID], f16,
                                  kind="ExternalOutput").ap()
        dbg_rec = nc.dram_tensor("dbg_rec", [nw, P, 4], f32,
                                 kind="ExternalOutput").ap()
        dbg_po = nc.dram_tensor("dbg_po", [nw, P, HID], f32,
                                kind="ExternalOutput").ap()
        dbg_vs = nc.dram_tensor("dbg_vs", [nw, P, 2], f32,
                                kind="ExternalOutput").ap()
        dbg_xde = nc.dram_tensor("dbg_xde", [nw, P, NET, HID], f16,
                                 kind="ExternalOutput").ap()

    with tile.TileContext(nc, trace_sim=False) as tc, ExitStack() as ctx:
        cpool = ctx.enter_context(tc.tile_pool(name="consts", bufs=1))
        bpool = ctx.enter_context(tc.tile_pool(name="build", bufs=3))
        psA = ctx.enter_context(tc.tile_pool(name="psA", bufs=g.slab_w,
                                             space="PSUM"))
        psB = ctx.enter_context(tc.tile_pool(name="psB", bufs=2, space="PSUM"))
        psC = ctx.enter_context(tc.tile_pool(name="psC", bufs=1, space="PSUM"))
        spool = ctx.enter_context(tc.tile_pool(name="slab", bufs=2))
        wpool = ctx.enter_context(tc.tile_pool(name="work", bufs=1))
        opool = ctx.enter_context(tc.tile_pool(name="oh", bufs=4))
        fpool = ctx.enter_context(tc.tile_pool(name="flush", bufs=2))

        # ---- constants ----------------------------------------------------
        ident = cpool.tile([P, P], f16)
        make_identity(nc, ident[:])
        iota16 = cpool.tile([P, P], mybir.dt.int16)
        nc.gpsimd.iota(iota16[:], pattern=[[1, P]], base=0, channel_multiplier=0)
        iota = cpool.tile([P, P], f16)
        nc.vector.tensor_copy(iota[:], iota16[:])
        ones_row = cpool.tile([1, P], f16)
        nc.vector.memset(ones_row[:], 1.0)
        eps_col = cpool.tile([P, 1], f32)
        nc.vector.memset(eps_col[:], EPS_LN)

        Ws = cpool.tile([HID, HID], f16)
        nc.sync.dma_start(Ws[:], W_src[:])
        Wd = cpool.tile([HID, HID], f16)
        nc.sync.dma_start(Wd[:], W_dst[:])
        Wo = cpool.tile([HID, HID], f16)
        nc.sync.dma_start(Wo[:], W_out[:])
        Wf = cpool.tile([HID, 2 * HID], f16)
        nc.sync.dma_start(Wf[:], W_film[:])
        emb_sb = cpool.tile([1, NET * HID], f16)
        nc.sync.dma_start(emb_sb[:], emb_eff[:])
        att_sb = cpool.tile([HID, H], f16)
        nc.sync.dma_start(att_sb[:], att_blk[:])
        task_sb = cpool.tile([HID, 1], f16)
        nc.sync.dma_start(task_sb[:], task[:])
        bfilm_sb = cpool.tile([1, 2 * HID], f32)
        nc.sync.dma_start(bfilm_sb[:], b_film[:])
        bout_sb = cpool.tile([1, HID], f32)
        nc.sync.dma_start(bout_sb[:], b_out[:])

        # node_own as [P, nw, HID]: partition p, window w -> node w*P+p
        node_own_sb = cpool.tile([P, nw, HID], f32, tag="nodeown")
        tail = npc - (npc // P) * P
        full_w = npc // P
        if tail:
            nc.vector.memset(node_own_sb[:, full_w, :], 0.0)
        if full_w:
            nc.sync.dma_start(
                node_own_sb[:, :full_w, :],
                node_own[:full_w * P, :].rearrange("(w p) h -> p w h", p=P))
        if tail:
            nc.sync.dma_start(node_own_sb[:tail, full_w, :],
                              node_own[full_w * P:, :])

        # ---- FiLM ---------------------------------------------------------
        ps_f = psB.tile([1, 2 * HID], f32, space="PSUM", tag="pt")
        nc.tensor.matmul(out=ps_f[:], lhsT=task_sb[:], rhs=Wf[:],
                         start=True, stop=True)
        film = cpool.tile([1, 2 * HID], f32)
        nc.vector.tensor_add(film[:], ps_f[:], bfilm_sb[:])
        gam_t = cpool.tile([1, HID], f32)
        nc.scalar.activation(gam_t[:], film[:, :HID], AF.Tanh)
        gam16 = cpool.tile([1, HID], f16)
        nc.vector.tensor_scalar(gam16[:], gam_t[:], 0.5, 1.0, OP.mult, OP.add)
        tmpb = cpool.tile([1, HID], f32)
        nc.vector.tensor_mul(tmpb[:], bout_sb[:], gam16[:])
        beta16 = cpool.tile([1, HID], f16)
        nc.vector.tensor_add(beta16[:], tmpb[:], film[:, HID:])
        ps_g = psB.tile([P, HID], f32, space="PSUM", tag="pt")
        nc.tensor.matmul(out=ps_g[:], lhsT=ones_row[:], rhs=gam16[:],
                         start=True, stop=True)
        gam_rep = cpool.tile([P, HID], f16)
        nc.vector.tensor_copy(gam_rep[:], ps_g[:])
        Wosc = cpool.tile([HID, HID], f16)
        nc.vector.tensor_mul(Wosc[:], Wo[:], gam_rep[:])
        ps_bt = psB.tile([P, HID], f32, space="PSUM", tag="pt")
        nc.tensor.matmul(out=ps_bt[:], lhsT=ones_row[:], rhs=beta16[:],
                         start=True, stop=True)
        beta_rep = cpool.tile([P, HID], f32)
        nc.vector.tensor_copy(beta_rep[:], ps_bt[:])

        if not sched["skip_norm"]:
            nw_dr = din("normw", [1, HID], f32)
            nb_dr = din("normb", [1, HID], f32)
            nw_sb = cpool.tile([1, HID], f32)
            nc.sync.dma_start(nw_sb[:], nw_dr[:])
            nb_sb = cpool.tile([1, HID], f32)
            nc.sync.dma_start(nb_sb[:], nb_dr[:])
            ones32 = cpool.tile([1, P], f32)
            nc.vector.memset(ones32[:], 1.0)
            ps_w = psB.tile([P, HID], f32, space="PSUM", tag="pt")
            nc.tensor.matmul(out=ps_w[:], lhsT=ones32[:], rhs=nw_sb[:],
                             start=True, stop=True)
            w_rep = cpool.tile([P, HID], f32)
            nc.vector.tensor_copy(w_rep[:], ps_w[:])
            ps_b = psB.tile([P, HID], f32, space="PSUM", tag="pt")
            nc.tensor.matmul(out=ps_b[:], lhsT=ones32[:], rhs=nb_sb[:],
                             start=True, stop=True)
            b_rep = cpool.tile([P, HID], f32)
            nc.vector.tensor_copy(b_rep[:], ps_b[:])

        Wd8 = cpool.tile([HID, NET * HID], f16)
        for t in range(NET):
            nc.vector.tensor_copy(Wd8[:, t * HID:(t + 1) * HID], Wd[:])

        # ---- xsrc table -----------------------------------------------------
        for i in range(g.n_pad // 512):
            nt = bpool.tile([HID, 512], f16, tag="nt")
            nc.sync.dma_start(nt[:], nodeT[:, i * 512:(i + 1) * 512])
            ps = psB.tile([P, 512], f32, space="PSUM", tag="pt")
            for j in range(4):
                nc.tensor.matmul(out=ps[:, j * HID:(j + 1) * HID],
                                 lhsT=nt[:, j * P:(j + 1) * P], rhs=Ws[:],
                                 start=True, stop=True, skip_group_check=True)
            xt = bpool.tile([P, 4, HID], f16, tag="xt")
            psv = ps[:].rearrange("p (s h) -> p s h", s=4)
            if i % 2 == 0:
                nc.scalar.activation(xt[:], psv, AF.Copy)
            else:
                nc.vector.tensor_copy(xt[:], psv)
            nc.sync.dma_start(
                xsrc_tab[i * 512:(i + 1) * 512, :]
                .rearrange("(s p) h -> p s h", p=P), xt[:])

        # ---- xd_ext table ---------------------------------------------------
        for w in range(nw):
            no16 = bpool.tile([P, HID], f16, tag="no16")
            nc.vector.tensor_copy(no16[:], node_own_sb[:, w, :])
            psT = psB.tile([P, P], f16, space="PSUM", tag="pt")
            nc.tensor.transpose(out=psT[:], in_=no16[:], identity=ident[:])
            noT = bpool.tile([HID, P], f16, tag="noT")
            nc.scalar.activation(noT[:], psT[:], AF.Copy)
            xde = bpool.tile([P, NET, HID], f16, tag="xde")
            for half in (0, 1):
                pst = psB.tile([P, 512], f32, space="PSUM", tag="pt")
                for t in range(4):
                    tt = half * 4 + t
                    nc.tensor.matmul(out=pst[:, t * HID:(t + 1) * HID],
                                     lhsT=noT[:],
                                     rhs=Wd8[:, tt * HID:(tt + 1) * HID],
                                     start=True, stop=False,
                                     skip_group_check=True)
                    nc.tensor.matmul(out=pst[:, t * HID:(t + 1) * HID],
                                     lhsT=ones_row[:],
                                     rhs=emb_sb[:, tt * HID:(tt + 1) * HID],
                                     start=False, stop=True,
                                     skip_group_check=True)
                pv = pst[:].rearrange("p (t h) -> p t h", t=4)
                if half == 0:
                    nc.scalar.activation(xde[:, :4, :], pv, AF.Copy)
                else:
                    nc.vector.tensor_copy(xde[:, 4:, :], pv)
            sl_i, wl_i = w // g.slab_w, w % g.slab_w
            nc.sync.dma_start(
                xd_tabs[sl_i][wl_i * P * NET:(wl_i + 1) * P * NET, :]
                .rearrange("(p t) h -> p t h", p=P), xde[:])
            if debug:
                nc.sync.dma_start(dbg_noT[w], noT[:])
                nc.sync.dma_start(dbg_no16[w], no16[:])
                nc.sync.dma_start(dbg_xde[w], xde[:])

        # fold beta_eff into the residual input (AFTER xd table build, which
        # needs the raw node embeddings): node_own_eff = node_own + beta
        nc.vector.tensor_add(
            node_own_sb[:], node_own_sb[:],
            beta_rep[:].unsqueeze(1).broadcast_to([P, nw, HID]))

        # ---- idx + dstr staging --------------------------------------------
        lo_sb = cpool.tile([P, lo_cols], mybir.dt.int16, tag="loidx")
        nc.sync.dma_start(lo_sb[:], lo_idx[:])
        hi_sb = cpool.tile([P, hi_cols], mybir.dt.int16, tag="hiidx")
        nc.sync.dma_start(hi_sb[:], hi_idx[:])
        xdi_sb = cpool.tile([P, xd_cols], mybir.dt.int16, tag="xdidx")
        nc.sync.dma_start(xdi_sb[:], xd_idx[:])
        dstr_sb = cpool.tile([P, total_chunks], f32, tag="dstr")
        nc.sync.dma_start(dstr_sb[:], dstr[:])

        off16 = {0: 0, 1: 0}
        qn = [0]

        def nextq():
            qn[0] = (qn[0] + 1) % 4
            return qn[0]

        # ---- edge slabs ----------------------------------------------------
        for s, sl in enumerate(sched["slabs"]):
            ws = sl["windows"]
            nwin = len(ws)
            chunks = sl["chunks"]
            C = len(chunks)
            c0 = sl["chunk0"]

            xs_t = spool.tile([P, C, HID], f16, tag="xs")
            xd_t = spool.tile([P, C, HID], f16, tag="xd")
            if "gather" in ablate:
                nc.vector.memset(xs_t[:], 0.0)
                nc.vector.memset(xd_t[:], 0.0)
            for h in (0, 1):
                base = g.split if h == 1 else 0
                idx_sb = lo_sb if h == 0 else hi_sb
                for (slot_off, n) in sl["calls"][h]:
                    if n == 0:
                        continue
                    if "gather" in ablate:
                        continue
                    nc.gpsimd.dma_gather(
                        out_ap=xs_t[:, slot_off:slot_off + n // P, :],
                        in_ap=xsrc_tab[base:g.n_pad, :],
                        idxs_ap=idx_sb[:, off16[h]:off16[h] + n // 16],
                        num_idxs=n, num_idxs_reg=n, elem_size=HID,
                        single_packet=(n <= 1024), queue_num=nextq(),
                    )
                    off16[h] += n // 16
            for (slot_off, n) in sl["calls"][2]:
                if "gather" in ablate:
                    continue
                nc.gpsimd.dma_gather(
                    out_ap=xd_t[:, slot_off:slot_off + n // P, :],
                    in_ap=xd_tabs[s][0:nwin * P * NET, :],
                    idxs_ap=xdi_sb[:, (c0 * P + slot_off * P) // 16:
                                   (c0 * P + slot_off * P + n) // 16],
                    num_idxs=n, num_idxs_reg=n, elem_size=HID,
                    single_packet=(n <= 1024), queue_num=nextq(),
                )

            comb = wpool.tile([P, C, HID], f16, tag="comb")
            if "compblend" not in ablate:
                nc.vector.tensor_add(comb[:], xs_t[:], xd_t[:])
                # leaky_relu(x) = max(x, 0.2x)
                lr_s = wpool.tile([P, C, HID], f16, tag="wgt")
                nc.vector.tensor_scalar_mul(lr_s[:], comb[:], 0.2)
                nc.vector.tensor_max(comb[:], comb[:], lr_s[:])

            win_ps = [psA.tile([P, 4 + HID], f32, space="PSUM", tag="win",
                                name=f"win{s}_{i}")
                      for i in range(nwin)]
            for wp in win_ps:
                nc.vector.memset(wp[:], 0.0)
            n_per_win = [0] * nwin
            for (wl, h, slot) in chunks:
                n_per_win[wl] += 1
            seen = [0] * nwin

            ex_ps = psC.tile([P, C, 4], f32, space="PSUM", tag="ex")
            combT = wpool.tile([HID, C, P], f16, tag="combT")
            if "logits" not in ablate:
                for g0 in range(0, C, 4):
                    gn = min(4, C - g0)
                    psT = psB.tile([P, 4, P], f16, space="PSUM", tag="pt")
                    for k in range(gn):
                        nc.tensor.transpose(out=psT[:, k, :],
                                            in_=comb[:, g0 + k, :],
                                            identity=ident[:])
                    if (g0 // 4) % 2 == 0:
                        nc.scalar.activation(combT[:, g0:g0 + gn, :],
                                             psT[:, :gn, :], AF.Copy)
                    else:
                        nc.vector.tensor_copy(combT[:, g0:g0 + gn, :],
                                              psT[:, :gn, :])
                    for k in range(gn):
                        slot = g0 + k
                        nc.tensor.matmul(out=ex_ps[:, slot, :],
                                         lhsT=combT[:, slot, :], rhs=att_sb[:],
                                         start=True, stop=True,
                                         skip_group_check=True)

            rhs_t = wpool.tile([P, C, 4 + HID], f16, tag="wgt")
            ex16 = rhs_t[:, :, 0:4]
            wgt = rhs_t[:, :, 4:4 + HID]
            if "logits" not in ablate:
                nc.scalar.activation(ex16, ex_ps[:], AF.Exp)
            if "weighted" not in ablate:
                nc.vector.tensor_mul(
                    wgt.rearrange("p c (h d) -> p c h d", h=4),
                    xs_t[:].rearrange("p c (h d) -> p c h d", h=4),
                    ex16.unsqueeze(3).broadcast_to([P, C, 4, HD]))

            for ohi, (wl, h, slot) in enumerate(chunks):
                if "scatter" in ablate:
                    continue
                oh = opool.tile([P, P], f16, tag="oh")
                eng = nc.vector if ohi % 2 == 0 else nc.gpsimd
                eng.tensor_scalar(
                    oh[:], iota[:], dstr_sb[:, c0 + slot:c0 + slot + 1], None,
                    OP.is_equal)
                last = seen[wl] == n_per_win[wl] - 1
                seen[wl] += 1
                nc.tensor.matmul(out=win_ps[wl][:], lhsT=oh[:],
                                 rhs=rhs_t[:, slot, :], start=False, stop=last,
                                 skip_group_check=True)

            if debug:
                nc.sync.dma_start(dbg_xs[s, :, :C, :], xs_t[:])
                nc.sync.dma_start(dbg_xd[s, :, :C, :], xd_t[:])
                nc.sync.dma_start(dbg_comb[s, :, :C, :], comb[:])
                nc.sync.dma_start(dbg_ex[s, :, :C, :], ex16)
                nc.sync.dma_start(dbg_wgt[s, :, :C, :], wgt)
                nc.sync.dma_start(dbg_combT[s, :, :C, :], combT[:])

            # ---- flush windows --------------------------------------------
            if "flush" in ablate:
                continue
            nwin_s = len(ws)
            vs_slab = fpool.tile([P, nwin_s], f32, tag="vs_slab",
                                 name=f"vss{s}")
            cen_l, y_l = [], []
            for wl, w in enumerate(ws):
                pw = win_ps[wl]
                if debug:
                    dbgw = fpool.tile([P, 4 + HID], f32, tag="dbgw")
                    nc.vector.tensor_copy(dbgw[:], pw[:])
                    nc.sync.dma_start(dbg_win[w], dbgw[:])
                sums = fpool.tile([P, 4], f32, tag="sums")
                nc.vector.tensor_scalar(sums[:], pw[:, 0:4], 1e-12, None,
                                        OP.max)
                rec = fpool.tile([P, 4], f32, tag="rec")
                nc.vector.reciprocal(rec[:], sums[:])
                aggn = fpool.tile([P, HID], f16, tag="aggn")
                nc.vector.tensor_mul(
                    aggn[:].rearrange("p (h d) -> p h d", h=4),
                    pw[:, 4:4 + HID].rearrange("p (h d) -> p h d", h=4),
                    rec[:].unsqueeze(2).broadcast_to([P, 4, HD]))
                if debug:
                    nc.sync.dma_start(dbg_aggn[w], aggn[:])
                    nc.sync.dma_start(dbg_rec[w], rec[:])
                psT = psB.tile([P, P], f16, space="PSUM", tag="pt")
                nc.tensor.transpose(out=psT[:], in_=aggn[:], identity=ident[:])
                aggT = fpool.tile([HID, P], f16, tag="aggT")
                nc.scalar.activation(aggT[:], psT[:], AF.Copy)
                po = psB.tile([P, HID], f32, space="PSUM", tag="pt")
                nc.tensor.matmul(out=po[:], lhsT=aggT[:], rhs=Wosc[:],
                                 start=True, stop=True)
                if debug:
                    dbgpo = fpool.tile([P, HID], f32, tag="dbgpo")
                    nc.vector.tensor_copy(dbgpo[:], po[:])
                    nc.sync.dma_start(dbg_po[w], dbgpo[:])
                y = fpool.tile([P, HID], f32, tag="y", name=f"y{s}_{wl}",
                               bufs=g.slab_w + 1)
                nc.vector.tensor_add(y[:], po[:], node_own_sb[:, w, :])
                mus = fpool.tile([P, 1], f32, tag="mus")
                nc.vector.tensor_reduce(mus[:], y[:], axis=mybir.AxisListType.X,
                                        op=OP.add)
                mu = fpool.tile([P, 1], f32, tag="mu")
                nc.vector.tensor_scalar(mu[:], mus[:], 1.0 / HID, None, OP.mult)
                cen = fpool.tile([P, HID], f32, tag="cen", name=f"cen{s}_{wl}",
                                 bufs=g.slab_w + 1)
                nc.vector.tensor_scalar(cen[:], y[:], mu[:], None, OP.subtract)
                cen_l.append(cen)
                if debug:
                    nc.sync.dma_start(dbg_y[w], y[:])
            # grouped Squares (one ACT table load), then one Sqrt for the slab
            for wl, w in enumerate(ws):
                sq = fpool.tile([P, HID], f16, tag="sq")
                nc.scalar.activation(sq[:], cen_l[wl][:], AF.Square,
                                     accum_out=vs_slab[:, wl:wl + 1])
            sd_s = fpool.tile([P, nwin_s], f32, tag="sd_s", name=f"sds{s}")
            nc.scalar.activation(sd_s[:], vs_slab[:], AF.Sqrt, bias=eps_col[:],
                                 scale=1.0 / HID)
            rstd_s = fpool.tile([P, nwin_s], f32, tag="rstd_s", name=f"rss{s}")
            nc.vector.reciprocal(rstd_s[:], sd_s[:])
            for wl, w in enumerate(ws):
                if debug:
                    nc.sync.dma_start(dbg_vs[w, :, 0:1], vs_slab[:, wl:wl + 1])
                yn = fpool.tile([P, HID], f32, tag="yn")
                nc.vector.tensor_scalar(yn[:], cen_l[wl][:],
                                        rstd_s[:, wl:wl + 1], None, OP.mult)
                if not sched["skip_norm"]:
                    nc.vector.tensor_mul(yn[:], yn[:], w_rep[:])
                    nc.vector.tensor_add(yn[:], yn[:], b_rep[:])
                rows = min(P, npc - w * P)
                nc.sync.dma_start(out[w * P:w * P + rows, :], yn[:rows, :])

    nc.compile()
    return nc


# ---------------------------------------------------------------------------
# Full-input entry point: shard, compile (cached), run SPMD on 8 cores,
# gather the output shards.
# ---------------------------------------------------------------------------
_CACHE = {}


def kernel(**inputs):
    N = int(np.asarray(inputs["node_embeddings"]).shape[0])
    n_cores = 8
    g = Geo(N=N, n_cores=n_cores, split=min(25000, N), slab_w=5)

    sched, in_maps = host_prep(g, **{k: np.asarray(v) for k, v in inputs.items()})

    key = (N, sched["total_chunks"], tuple(int(x) for x in sched["caps"].ravel()),
           sched["skip_norm"])
    if key not in _CACHE:
        _CACHE[key] = build_program(g, sched)
    nc = _CACHE[key]

    from concourse.bass_utils import run_bass_kernel_spmd
    res = run_bass_kernel_spmd(nc, in_maps, core_ids=list(range(n_cores)))
    out = np.concatenate([res.results[c]["out"] for c in range(n_cores)], axis=0)
    return out.astype(np.float32)

